# revision 1
# baseline (speedup 1.0000x reference)
"""CTC loss (keras ctc_batch_cost semantics) on 8 Trainium2 NeuronCores.

Strategy
--------
Pure data parallel over the batch: 8 cores x 64 examples each; no collectives.

The CTC forward recursion runs in the probability domain (not log space):
    alpha_t[s] = (alpha_{t-1}[s] + alpha_{t-1}[s-1] + allow[s]*alpha_{t-1}[s-2])
                 * p_t[ext[s]]
with a rescale every RENORM steps that renormalizes the per-example total to
K = 2**100, keeping the state-profile peak near the top of the fp32 exponent
range so ~150 nats of spread below the peak stay representable (a plain
renorm-to-1 loses ~1% of the probability mass to underflow; log-space per-step
logaddexp is far too slow on this hardware).  The per-step sums C_t come for
free from the final multiply's fused accumulator; the host reassembles
    loss = -(log(alpha_T[S-1] + alpha_T[S-2]) - logK + sum_k log(C_k/K))
in float64.  Storage is bf16 (DVE computes in fp32 internally; bf16 keeps the
fp32 exponent range), measured end-to-end max rel err ~1.1e-4.

Device layout: batch in partitions (64 rows), states along the free dim with
2 zero-pad columns so the s-1/s-2 shifts are plain AP offsets.  Each step is
4 fused scalar_tensor_tensor ops on the vector engine.  The gathered
probability tensor P[b, t, s] = y_pred[b, t, ext[b, s]] + eps is built on the
host (per-partition gathers are not expressible on-device: indirect_copy
shares its index stream across each 16-partition group) and streamed to the
device in double-buffered time chunks; P rows are padded to an even length so
per-step slices stay 4-byte aligned for the DVE 2x bf16 mode.
"""

import ml_dtypes
import numpy as np

import concourse.bacc as bacc
import concourse.bass as bass
import concourse.tile as tile
from concourse import mybir
from concourse.bass_utils import run_bass_kernel_spmd

B, T, C, L = 512, 512, 128, 64
S = 2 * L + 1
SP = S + 1             # P row padded to even length so per-step offsets stay 4B-aligned
BLANK = C - 1
EPS = 1e-7
NCORES = 8
BPC = B // NCORES
RENORM = 4
K = float(2.0 ** 100)
LOG_K = 100.0 * float(np.log(2.0))

F32 = mybir.dt.float32
BF16 = mybir.dt.bfloat16
MULT = mybir.AluOpType.mult
ADD = mybir.AluOpType.add


def build_nc(T_=T, TB=64, bpc=BPC, renorm=RENORM):
    nc = bacc.Bacc(
        "TRN2", target_bir_lowering=False, debug=False, num_devices=NCORES
    )
    P = nc.dram_tensor("P", [bpc, T_, SP], BF16, kind="ExternalInput")
    M = nc.dram_tensor("M", [bpc, S - 2], BF16, kind="ExternalInput")
    XF = nc.dram_tensor("XF", [bpc, 2], F32, kind="ExternalOutput")
    CS = nc.dram_tensor("CS", [bpc, T_], F32, kind="ExternalOutput")

    Pap, Map, XFap, CSap = P.ap(), M.ap(), XF.ap(), CS.ap()
    nchunks = T_ // TB

    with tile.TileContext(nc) as tc:
        with (
            tc.tile_pool(name="persist", bufs=1) as pers,
            tc.tile_pool(name="pchunks", bufs=2) as pp,
        ):
            X = pers.tile([bpc, S + 2], BF16)
            W = pers.tile([bpc, S], BF16)
            G = pers.tile([bpc, S - 2], BF16)
            m = pers.tile([bpc, S - 2], BF16)
            Cs = pers.tile([bpc, T_], F32)
            rc = pers.tile([bpc, 1], F32)
            xf32 = pers.tile([bpc, 2], F32)

            nc.vector.memset(X, 0.0)
            nc.vector.memset(Cs, 0.0)
            nc.sync.dma_start(out=m, in_=Map)

            for k in range(nchunks):
                pch = pp.tile([bpc, TB, SP], BF16, tag="pch")
                nc.sync.dma_start(out=pch, in_=Pap[:, k * TB : (k + 1) * TB, :])

                for i in range(TB):
                    tau = k * TB + i
                    pt = pch[:, i, 0:S]
                    if tau == 0:
                        nc.vector.tensor_scalar_mul(X[:, 2:4], pt[:, 0:2], K)
                        nc.vector.tensor_reduce(
                            Cs[:, 0:1], X[:, 2:4], axis=mybir.AxisListType.X, op=ADD
                        )
                        continue
                    renorm_step = tau % renorm == 0
                    feeds_renorm = (tau + 1) % renorm == 0 and tau + 1 < T_
                    if renorm_step:
                        nc.vector.reciprocal(rc, Cs[:, tau - 1 : tau])
                        nc.vector.tensor_scalar_mul(rc, rc, K)
                    # plain tensor_tensor where no scalar/accum is needed: TT has
                    # a bf16 2x_1p uop on HW; scalar_tensor_tensor may not.
                    nc.vector.tensor_add(W, X[:, 1 : S + 1], X[:, 2 : S + 2])
                    nc.vector.tensor_mul(G, X[:, 2:S], m)
                    nc.vector.tensor_add(W[:, 2:S], G, W[:, 2:S])
                    if renorm_step or feeds_renorm:
                        nc.vector.scalar_tensor_tensor(
                            X[:, 2 : S + 2],
                            W,
                            rc[:, :] if renorm_step else 1.0,
                            pt,
                            op0=MULT,
                            op1=MULT,
                            accum_out=Cs[:, tau : tau + 1] if feeds_renorm else None,
                        )
                    else:
                        nc.vector.tensor_mul(X[:, 2 : S + 2], W, pt)

            # upconvert the two final states to f32 for output
            nc.vector.tensor_copy(xf32, X[:, S : S + 2])
            nc.sync.dma_start(out=XFap, in_=xf32)
            nc.sync.dma_start(out=CSap, in_=Cs)

    nc.compile()
    return nc


def host_build_inputs(y_true, y_pred, T_=T):
    y_true = np.asarray(y_true).astype(np.int64)
    y_pred = np.asarray(y_pred).astype(np.float32)
    Bn = y_true.shape[0]
    p = y_pred + np.float32(EPS)
    ext = np.full((Bn, S), BLANK, dtype=np.int64)
    ext[:, 1::2] = y_true
    allow = np.zeros((Bn, S), dtype=bool)
    allow[:, 2:] = (ext[:, 2:] != BLANK) & (ext[:, 2:] != ext[:, :-2])
    P_full = np.zeros((Bn, T_, SP), dtype=ml_dtypes.bfloat16)
    P_full[:, :, :S] = np.take_along_axis(
        p[:, :T_, :], np.broadcast_to(ext[:, None, :], (Bn, T_, S)), axis=2
    ).astype(ml_dtypes.bfloat16)
    M_full = np.ascontiguousarray(allow[:, 2:].astype(ml_dtypes.bfloat16))
    return P_full, M_full


def host_finalize(XF, CS, T_=T, renorm=RENORM):
    fin = XF[:, 0].astype(np.float64) + XF[:, 1].astype(np.float64)
    corr = -np.log(np.float64(K))
    for k in range(1, T_ // renorm):
        corr = corr + (
            np.log(CS[:, k * renorm - 1].astype(np.float64)) - np.log(np.float64(K))
        )
    return (-(np.log(fin) + corr))[:, None].astype(np.float32)


TRACE = False
LAST_RESULT = None
LAST_EXEC_S = None
_NC_CACHE = None


def kernel(y_true, y_pred):
    global LAST_RESULT, LAST_EXEC_S, _NC_CACHE
    import time as _time

    P_full, M_full = host_build_inputs(y_true, y_pred)
    if _NC_CACHE is None:
        _NC_CACHE = build_nc()
    nc = _NC_CACHE
    in_maps = [
        {
            "P": np.ascontiguousarray(P_full[c * BPC : (c + 1) * BPC]),
            "M": np.ascontiguousarray(M_full[c * BPC : (c + 1) * BPC]),
        }
        for c in range(NCORES)
    ]
    t0 = _time.time()
    res = run_bass_kernel_spmd(
        nc, in_maps, core_ids=list(range(NCORES)), trace=TRACE
    )
    LAST_EXEC_S = _time.time() - t0
    LAST_RESULT = res
    out = np.empty((B, 1), dtype=np.float32)
    for c in range(NCORES):
        r = res.results[c]
        out[c * BPC : (c + 1) * BPC] = host_finalize(r["XF"], r["CS"])
    return out



# revision 4
# speedup vs baseline: 3.0600x; 3.0600x over previous
"""CTC loss (keras ctc_batch_cost semantics) on 8 Trainium2 NeuronCores.

Strategy (v2: time-major row scans)
-----------------------------------
Pure data parallel over the batch: 8 cores x 64 examples each; no collectives.

The CTC forward recursion is reorganized state-major -> time-major: for each
extended-label state s (row), the recursion over time

    alpha_s(t) = (alpha_{s-1}(t-1) + m_s * alpha_{s-2}(t-1) + alpha_s(t-1)) * p_s(t)

is a first-order linear recurrence along t, which maps to ONE hardware
tensor_tensor_scan instruction (state = (data0 + state) * data1, fp32 carry)
over [64 examples (partitions), T=512 (free)].  The whole forward pass is a
serial chain of 129 scans (one per row) plus one scalar_tensor_tensor prep op
per interior label row (W = alpha_{s-1} + m * alpha_{s-2}; the per-example
skip mask m rides in as the per-partition scalar).  Blank rows need no prep
(data0 = previous row directly) and all share a single p_blank stream, so the
gathered probability tensor shrinks to 65 rows (blank + 64 labels).

Numerics: the scan runs in the probability domain over all 512 steps, which
spans ~2300 nats.  The host pre-scales each time column of P by exp(chat(b,t)),
where chat = -log(mean gathered p) - log(trellis path-count ratio rho(t)); the
rho curve is pure CTC-trellis combinatorics (input independent), computed at
runtime.  This keeps the running scan values within ~[-15, +55] nats of 1.0
(measured), safely inside bf16/fp32 exponent range; the host subtracts
sum_t chat exactly in fp64, so only range placement (not correctness) depends
on the estimator.  Storage is bf16; the scan carry is fp32 internally.
Measured end-to-end max rel err ~5.6e-4.
"""

import ml_dtypes
import numpy as np

import concourse.bacc as bacc
import concourse.bass as bass
import concourse.tile as tile
from concourse import mybir
from concourse.bass_utils import run_bass_kernel_spmd

B, T, C, L = 512, 512, 128, 64
S = 2 * L + 1
BLANK = C - 1
EPS = 1e-7
NCORES = 8
BPC = B // NCORES
K0 = 45.0  # log placement of the t=0 column

F32 = mybir.dt.float32
BF16 = mybir.dt.bfloat16
MULT = mybir.AluOpType.mult
ADD = mybir.AluOpType.add

PL_GROUP = 16  # label rows per DMA group


def build_nc():
    nc = bacc.Bacc(
        "TRN2", target_bir_lowering=False, debug=False, num_devices=NCORES
    )
    PBd = nc.dram_tensor("PB", [BPC, T], BF16, kind="ExternalInput")
    PLd = nc.dram_tensor("PL", [BPC, L, T], BF16, kind="ExternalInput")
    Md = nc.dram_tensor("M", [BPC, L - 1], BF16, kind="ExternalInput")
    XFd = nc.dram_tensor("XF", [BPC, 2], F32, kind="ExternalOutput")

    PBap, PLap, Map, XFap = PBd.ap(), PLd.ap(), Md.ap(), XFd.ap()
    ngroups = L // PL_GROUP

    with tile.TileContext(nc) as tc:
        with (
            tc.tile_pool(name="pers", bufs=1) as pers,
            tc.tile_pool(name="plg", bufs=2) as plg,
        ):
            PBt = pers.tile([BPC, T], BF16)
            Mt = pers.tile([BPC, L - 1], BF16)
            R = pers.tile([BPC, 4, T + 1], BF16)
            Z = pers.tile([BPC, T], BF16)
            W = pers.tile([BPC, 2, T], BF16)
            XFt = pers.tile([BPC, 2], F32)

            nc.sync.dma_start(out=PBt, in_=PBap)
            nc.sync.dma_start(out=Mt, in_=Map)
            nc.vector.memset(R[:, :, 0:1], 0.0)
            nc.vector.memset(Z, 0.0)

            plt = [None] * ngroups

            def pl_row(i):
                return plt[i // PL_GROUP][:, i % PL_GROUP, 0:T]

            # prefetch group 0
            plt[0] = plg.tile([BPC, PL_GROUP, T], BF16, tag="plg", name="plg0")
            nc.sync.dma_start(
                out=plt[0], in_=PLap[:, 0:PL_GROUP, :]
            )

            for s in range(S):
                i = (s - 1) // 2  # label index for odd s
                if s % 2 == 1 and i % PL_GROUP == 0:
                    g = i // PL_GROUP
                    if g + 1 < ngroups and plt[g + 1] is None:
                        plt[g + 1] = plg.tile(
                            [BPC, PL_GROUP, T], BF16, tag="plg", name=f"plg{g + 1}"
                        )
                        nc.sync.dma_start(
                            out=plt[g + 1],
                            in_=PLap[:, (g + 1) * PL_GROUP : (g + 2) * PL_GROUP, :],
                        )
                out = R[:, s % 4, 1 : T + 1]
                if s == 0:
                    nc.vector.tensor_tensor_scan(
                        out, Z, PBt, 1.0, op0=ADD, op1=MULT
                    )
                elif s == 1:
                    nc.vector.tensor_tensor_scan(
                        out, R[:, 0, 0:T], pl_row(0), 1.0, op0=ADD, op1=MULT
                    )
                elif s % 2 == 0:
                    nc.vector.tensor_tensor_scan(
                        out, R[:, (s - 1) % 4, 0:T], PBt, 0.0, op0=ADD, op1=MULT
                    )
                else:
                    w = W[:, i % 2, :]
                    nc.vector.scalar_tensor_tensor(
                        w,
                        R[:, (s - 2) % 4, 0:T],
                        Mt[:, i - 1 : i],
                        R[:, (s - 1) % 4, 0:T],
                        op0=MULT,
                        op1=ADD,
                    )
                    nc.vector.tensor_tensor_scan(
                        out, w, pl_row(i), 0.0, op0=ADD, op1=MULT
                    )

            nc.vector.tensor_copy(XFt[:, 0:1], R[:, (S - 2) % 4, T : T + 1])
            nc.vector.tensor_copy(XFt[:, 1:2], R[:, (S - 1) % 4, T : T + 1])
            nc.sync.dma_start(out=XFap, in_=XFt)

    nc.compile()
    return nc


def _trellis_logrho():
    """log of per-step path-count growth of the CTC trellis (input indep.)."""
    N = np.zeros(S)
    N[0] = 1.0
    N[1] = 1.0
    logrho = np.zeros(T)
    for t in range(1, T):
        n1 = np.concatenate([[0.0], N[:-1]])
        n2 = np.concatenate([[0.0, 0.0], N[:-2]])
        n2[0::2] = 0.0  # blank states take no skip transition
        Nn = N + n1 + n2
        tot = Nn.sum()
        logrho[t] = np.log(tot)
        N = Nn / tot
    return logrho


def host_build_inputs(y_true, y_pred):
    y_true = np.asarray(y_true).astype(np.int64)
    y_pred = np.asarray(y_pred).astype(np.float64)
    Bn = y_true.shape[0]
    Pb = y_pred[:, :, BLANK] + EPS  # [B, T]
    Pl = (
        np.take_along_axis(y_pred, y_true[:, None, :], axis=2) + EPS
    )  # [B, T, L]
    m = (y_true[:, 1:] != y_true[:, :-1]).astype(np.float64)  # [B, L-1]

    q = (65.0 * Pb + Pl.sum(2)) / 129.0
    chat = -np.log(q) - _trellis_logrho()[None, :]
    chat[:, 0] = K0
    scale = np.exp(chat)

    bf = ml_dtypes.bfloat16
    PB = (Pb * scale).astype(bf)  # [B, T]
    PL = np.ascontiguousarray(
        (Pl * scale[:, :, None]).transpose(0, 2, 1).astype(bf)
    )  # [B, L, T]
    M = m.astype(bf)  # [B, L-1]
    Csum = chat.sum(1)  # [B] fp64, exact bookkeeping
    return PB, PL, M, Csum


TRACE = False
LAST_RESULT = None
LAST_EXEC_S = None
_NC_CACHE = None


def kernel(y_true, y_pred):
    global LAST_RESULT, LAST_EXEC_S, _NC_CACHE
    import time as _time

    PB, PL, M, Csum = host_build_inputs(y_true, y_pred)
    if _NC_CACHE is None:
        _NC_CACHE = build_nc()
    nc = _NC_CACHE
    in_maps = [
        {
            "PB": np.ascontiguousarray(PB[c * BPC : (c + 1) * BPC]),
            "PL": np.ascontiguousarray(PL[c * BPC : (c + 1) * BPC]),
            "M": np.ascontiguousarray(M[c * BPC : (c + 1) * BPC]),
        }
        for c in range(NCORES)
    ]
    t0 = _time.time()
    res = run_bass_kernel_spmd(
        nc, in_maps, core_ids=list(range(NCORES)), trace=TRACE
    )
    LAST_EXEC_S = _time.time() - t0
    LAST_RESULT = res
    out = np.empty((B, 1), dtype=np.float32)
    for c in range(NCORES):
        xf = res.results[c]["XF"].astype(np.float64)
        fin = xf[:, 0] + xf[:, 1]
        sl = slice(c * BPC, (c + 1) * BPC)
        out[sl, 0] = (-(np.log(fin) - Csum[sl])).astype(np.float32)
    return out


# revision 13
# speedup vs baseline: 3.6286x; 1.1858x over previous
"""CTC loss (keras ctc_batch_cost semantics) on 8 Trainium2 NeuronCores.

Strategy (v2: time-major row scans)
-----------------------------------
Pure data parallel over the batch: 8 cores x 64 examples each; no collectives.

The CTC forward recursion is reorganized state-major -> time-major: for each
extended-label state s (row), the recursion over time

    alpha_s(t) = (alpha_{s-1}(t-1) + m_s * alpha_{s-2}(t-1) + alpha_s(t-1)) * p_s(t)

is a first-order linear recurrence along t, which maps to ONE hardware
tensor_tensor_scan instruction (state = (data0 + state) * data1, fp32 carry)
over [64 examples (partitions), T=512 (free)].  The whole forward pass is a
serial chain of 129 scans (one per row) plus one scalar_tensor_tensor prep op
per interior label row (W = alpha_{s-1} + m * alpha_{s-2}; the per-example
skip mask m rides in as the per-partition scalar).  Blank rows need no prep
(data0 = previous row directly) and all share a single p_blank stream, so the
gathered probability tensor shrinks to 65 rows (blank + 64 labels).

Numerics: the scan runs in the probability domain over all 512 steps, which
spans ~2300 nats.  The host pre-scales each time column of P by exp(chat(b,t)),
where chat = -log(mean gathered p) - log(trellis path-count ratio rho(t)); the
rho curve is pure CTC-trellis combinatorics (input independent), computed at
runtime.  This keeps the running scan values within ~[-15, +55] nats of 1.0
(measured), safely inside bf16/fp32 exponent range; the host subtracts
sum_t chat exactly in fp64, so only range placement (not correctness) depends
on the estimator.  Storage is bf16; the scan carry is fp32 internally.
Measured end-to-end max rel err ~5.6e-4.
"""

import ml_dtypes
import numpy as np

import concourse.bacc as bacc
import concourse.bass as bass
import concourse.tile as tile
from concourse import mybir
from concourse.bass_utils import run_bass_kernel_spmd

B, T, C, L = 512, 512, 128, 64
S = 2 * L + 1
BLANK = C - 1
EPS = 1e-7
NCORES = 8
BPC = B // NCORES
K0 = 45.0  # log placement of the t=0 column

F32 = mybir.dt.float32
BF16 = mybir.dt.bfloat16
MULT = mybir.AluOpType.mult
ADD = mybir.AluOpType.add

PL_GROUP = 16  # label rows per DMA group


def _t0(s):
    return s // 2  # first t where alpha(t, s) can be nonzero


def _t1(s):
    # last needed t + 1: (t, s) must still reach states {S-2, S-1} by T-1
    return T - (S - 1 - s) // 2


MAXLEN = max(_t1(s) - _t0(s) for s in range(S))  # 449


def build_nc():
    nc = bacc.Bacc(
        "TRN2", target_bir_lowering=False, debug=False, num_devices=NCORES
    )
    PBd = nc.dram_tensor("PB", [BPC, T], BF16, kind="ExternalInput")
    PLd = nc.dram_tensor("PL", [BPC, L, T], BF16, kind="ExternalInput")
    Md = nc.dram_tensor("M", [BPC, L - 1], BF16, kind="ExternalInput")
    XFd = nc.dram_tensor("XF", [BPC, 2], F32, kind="ExternalOutput")

    PBap, PLap, Map, XFap = PBd.ap(), PLd.ap(), Md.ap(), XFd.ap()
    # first groups small so the s=1.. scans can start early
    group_sizes = [2, 6, 8, 16, 16, 16]
    group_starts = [sum(group_sizes[:g]) for g in range(len(group_sizes))]

    with tile.TileContext(nc) as tc:
        with tc.tile_pool(name="pers", bufs=1) as pers:
            plg = pers  # PL group tiles fit in SBUF persistently (~64KB/part)
            PBt = pers.tile([BPC, T], BF16)
            Mt = pers.tile([BPC, L - 1], BF16)
            # skewed rows: col 0 = permanent zero pad; col 1+k = alpha(t0(s)+k)
            R = pers.tile([BPC, 4, MAXLEN + 1], BF16)
            Z = pers.tile([BPC, MAXLEN], BF16)
            W = pers.tile([BPC, 2, MAXLEN // 2 + 1], BF16)
            W2 = pers.tile([BPC, 2, MAXLEN // 2 + 1], BF16)
            XFt = pers.tile([BPC, 2], F32)

            nc.sync.dma_start(out=PBt, in_=PBap)

            plt = [None] * len(group_sizes)
            plt[0] = plg.tile([BPC, group_sizes[0], T], BF16, name="plg0")
            nc.sync.dma_start(out=plt[0], in_=PLap[:, 0 : group_sizes[0], :])
            nc.sync.dma_start(out=Mt, in_=Map)

            nc.vector.memset(R[:, :, 0:1], 0.0)
            nc.vector.memset(Z, 0.0)

            def pl_row(i):
                g = next(
                    g for g in range(len(group_sizes))
                    if group_starts[g] <= i < group_starts[g] + group_sizes[g]
                )
                return plt[g][:, i - group_starts[g], :]

            def prefetch(i):
                # issue group g+1's DMA when starting the first row of group g
                for g in range(len(group_sizes)):
                    if i == group_starts[g] and g + 1 < len(group_sizes):
                        if plt[g + 1] is None:
                            plt[g + 1] = plg.tile(
                                [BPC, group_sizes[g + 1], T],
                                BF16,
                                name=f"plg{g + 1}",
                            )
                            nc.sync.dma_start(
                                out=plt[g + 1],
                                in_=PLap[
                                    :,
                                    group_starts[g + 1] : group_starts[g + 1]
                                    + group_sizes[g + 1],
                                    :,
                                ],
                            )

            # Each row's ops are split at stored column h into two halves and
            # the halves of adjacent rows are interleaved so no DVE op
            # directly follows the op it depends on (hides the ~95ns
            # dependent-dispatch bubble).  Emission order per row r:
            #   odd r:  [P0_r, B1_{r-1}, L0_r]     (P = prep stt, B/L = scans)
            #   even r: [P1_{r-1}, B0_r, L1_{r-1}]
            # The second half chains through the scan's `initial` carry
            # (= the row's stored col h-1).
            def _h(s):
                return 1 + (_t1(s) - _t0(s)) // 2

            def halfrange(s, half):
                ln = _t1(s) - _t0(s)
                return (1, _h(s)) if half == 0 else (_h(s), 1 + ln)

            def emit_prep(s, half):
                # W[c] = R_{s-1}[col c-1] + m_i * R_{s-2}[col c]
                a, b = halfrange(s, half)
                i = (s - 1) // 2
                w = (W if half == 0 else W2)[:, i % 2, 0 : b - a]
                nc.vector.scalar_tensor_tensor(
                    w,
                    R[:, (s - 2) % 4, a:b],
                    Mt[:, i - 1 : i],
                    R[:, (s - 1) % 4, a - 1 : b - 1],
                    op0=MULT,
                    op1=ADD,
                )
                return w

            def emit_scan(s, half, w=None):
                a, b = halfrange(s, half)
                out = R[:, s % 4, a:b]
                if half == 1:
                    init = R[:, s % 4, a - 1 : a]
                else:
                    init = 1.0 if s <= 1 else 0.0
                if s % 2 == 1 and s >= 3:
                    d0 = w
                elif s == 0:
                    d0 = Z[:, a - 1 : b - 1]
                elif s == 1:
                    d0 = R[:, 0, a - 1 : b - 1]
                else:
                    d0 = R[:, (s - 1) % 4, a:b]
                if s % 2 == 0:
                    d1 = PBt[:, _t0(s) + a - 1 : _t0(s) + b - 1]
                else:
                    d1 = pl_row((s - 1) // 2)[
                        :, _t0(s) + a - 1 : _t0(s) + b - 1
                    ]
                nc.vector.tensor_tensor_scan(out, d0, d1, init, op0=ADD, op1=MULT)

            w0 = {}  # half-0 prep APs awaiting their scan
            w1 = {}
            for r in range(S):
                if r % 2 == 1:
                    prefetch((r - 1) // 2)
                    if r >= 3:
                        w0[r] = emit_prep(r, 0)  # P0_r
                    if r == 1:
                        emit_scan(1, 0)  # L0_1 (no prep)
                        emit_scan(0, 1)  # B1_0
                    else:
                        emit_scan(r - 1, 1)  # B1_{r-1}
                        emit_scan(r, 0, w0.pop(r))  # L0_r
                else:
                    if r >= 4:
                        w1[r - 1] = emit_prep(r - 1, 1)  # P1_{r-1}
                    emit_scan(r, 0)  # B0_r
                    if r == 2:
                        emit_scan(1, 1)  # L1_1 (no prep)
                    elif r >= 4:
                        emit_scan(r - 1, 1, w1.pop(r - 1))  # L1_{r-1}
            emit_scan(S - 1, 1)  # B1_{S-1}

            ln2 = _t1(S - 2) - _t0(S - 2)
            ln1 = _t1(S - 1) - _t0(S - 1)
            nc.vector.tensor_copy(XFt[:, 0:1], R[:, (S - 2) % 4, ln2 : ln2 + 1])
            nc.vector.tensor_copy(XFt[:, 1:2], R[:, (S - 1) % 4, ln1 : ln1 + 1])
            nc.sync.dma_start(out=XFap, in_=XFt)

    nc.compile()
    return nc


def _trellis_logrho():
    """log of per-step path-count growth of the CTC trellis (input indep.)."""
    N = np.zeros(S)
    N[0] = 1.0
    N[1] = 1.0
    logrho = np.zeros(T)
    for t in range(1, T):
        n1 = np.concatenate([[0.0], N[:-1]])
        n2 = np.concatenate([[0.0, 0.0], N[:-2]])
        n2[0::2] = 0.0  # blank states take no skip transition
        Nn = N + n1 + n2
        tot = Nn.sum()
        logrho[t] = np.log(tot)
        N = Nn / tot
    return logrho


def host_build_inputs(y_true, y_pred):
    y_true = np.asarray(y_true).astype(np.int64)
    y_pred = np.asarray(y_pred).astype(np.float64)
    Bn = y_true.shape[0]
    Pb = y_pred[:, :, BLANK] + EPS  # [B, T]
    Pl = (
        np.take_along_axis(y_pred, y_true[:, None, :], axis=2) + EPS
    )  # [B, T, L]
    m = (y_true[:, 1:] != y_true[:, :-1]).astype(np.float64)  # [B, L-1]

    q = (65.0 * Pb + Pl.sum(2)) / 129.0
    chat = -np.log(q) - _trellis_logrho()[None, :]
    chat[:, 0] = K0
    scale = np.exp(chat)

    bf = ml_dtypes.bfloat16
    PB = (Pb * scale).astype(bf)  # [B, T]
    PL = np.ascontiguousarray(
        (Pl * scale[:, :, None]).transpose(0, 2, 1).astype(bf)
    )  # [B, L, T]
    M = m.astype(bf)  # [B, L-1]
    Csum = chat.sum(1)  # [B] fp64, exact bookkeeping
    return PB, PL, M, Csum


TRACE = False
LAST_RESULT = None
LAST_EXEC_S = None
_NC_CACHE = None


def kernel(y_true, y_pred):
    global LAST_RESULT, LAST_EXEC_S, _NC_CACHE
    import time as _time

    PB, PL, M, Csum = host_build_inputs(y_true, y_pred)
    if _NC_CACHE is None:
        _NC_CACHE = build_nc()
    nc = _NC_CACHE
    in_maps = [
        {
            "PB": np.ascontiguousarray(PB[c * BPC : (c + 1) * BPC]),
            "PL": np.ascontiguousarray(PL[c * BPC : (c + 1) * BPC]),
            "M": np.ascontiguousarray(M[c * BPC : (c + 1) * BPC]),
        }
        for c in range(NCORES)
    ]
    t0 = _time.time()
    res = run_bass_kernel_spmd(
        nc, in_maps, core_ids=list(range(NCORES)), trace=TRACE
    )
    LAST_EXEC_S = _time.time() - t0
    LAST_RESULT = res
    out = np.empty((B, 1), dtype=np.float32)
    for c in range(NCORES):
        xf = res.results[c]["XF"].astype(np.float64)
        fin = xf[:, 0] + xf[:, 1]
        sl = slice(c * BPC, (c + 1) * BPC)
        out[sl, 0] = (-(np.log(fin) - Csum[sl])).astype(np.float32)
    return out
